# revision 62
# baseline (speedup 1.0000x reference)
"""DequantingLinear Trainium2 kernel — transposed-codes redesign.

y = x @ W^T + b where W = (w_q - 128) * w_scales (GGML Q8_0-style, block=32),
b = (b_q - 128) * b_scales.

Sharding: column-parallel over out_features across 8 cores (1536 rows of W
per core).  Design vs the first-generation kernel (~89 us):

1. The int32 codes carry one useful byte; the host repacks them (pure
   storage change, values identical): uint8 for half the k-tiles, float16
   (still the raw 0..255 code values) for the other half — the dominant
   HBM stream shrinks ~3x.
2. The host pre-TRANSPOSES the code matrix to [in, out] layout (layout
   only, like the x transpose), so the PE consumes dequantized tiles
   directly: no PE transposes, no PSUM evacuation traffic.  PE work per
   core collapses to 24 k-tiles x 3 N=512 matmuls + a few extras.
3. Block-to-partition permutation: a k-tile of 128 i-rows normally spans 4
   quant blocks, making the scale operand a cross-partition gather.  We
   instead permute which i lands on which (k-tile, partition) slot so each
   lane's scale is constant per tile and the scale operand is an ordinary
   [128, 1536] step-1 fp16 tile: 16 "L0" k-tiles (lane p -> block p mod 96)
   and 8 "L1" k-tiles (lane p -> block 32 + p mod 64), covering each
   (block, j) exactly once.  x is permuted identically on the host.
4. Dequant engine split (HW-measured: DVE tensor_tensor 2x_1p 938ns/tile,
   ACT cast 1.56us, DVE STT 1x 1.74us; GPSIMD elementwise shares SBUF
   ports with DVE — running it measured DVE TTs at 3.7us — so GPSIMD only
   triggers SWDGE DMAs here):
   - "A" tiles (uint8): ACT activation(Copy, bias=-128) -> fp16, then DVE
     2x tensor_mul by the scale tile.
   - "F" tiles (float16 raw codes): DVE 2x tensor_mul directly; the
     missing -128 shift is restored by 3 matmuls per scale layout with
     lhsT = -128 * sum of those tiles' x slices (matmul is linear in lhsT
     and the rhs — the scale tile — is shared).  The sum is reduced in
     fp32 and split into exact hi+lo fp16 parts (two matmul sets).
5. Bias: ACT casts (bq-128)->fp16 (otherwise idle), DVE multiplies by the
   host-REPLICATED per-block scales (plain 1-D APs; a (1,48,32) sub-dim AP
   measured ~3x slower), added via K=1 matmuls against a ones row.
6. DMA choreography: the SP HWDGE ring carries [scales-L0, then u8/f16
   code chunks interleaved, y] — exactly its 8 completion lanes; GPSIMD
   SWDGE (own lane space) carries xt, bias bytes, and scales-L1 (needed
   late).  scales-L0 first + interleaved chunk types keep ACT (casts) and
   DVE (multiplies) fed from ~8us; partition-major DRAM layouts keep every
   transfer at 128 large descriptors (a [3200,64] rearranged xt DMA
   measured 15.6us of descriptor generation; partition-major is ~1us).

Two TRN2 toolchain quirks are handled explicitly (see _strip_self_waits
and _patch_drain_split): several instruction structs encode at most ONE
semaphore wait (walrus "Too many sync wait commands"), and the kernel-tail
drain's global-clock waits are pre-spread across SP nops.  Producers are
arranged to carry exactly one wait (one-buffer-per-tile pools, head
absorbers for the scale/bias tiles); a post-pass drops provably redundant
waits.  Barrier semaphores are reset between rounds, so the post-pass only
dedupes waits on monotonic sems (engine clocks + DMA lanes) — deduping a
barrier wait deadlocks the kernel (found the hard way).
"""

import sys

import numpy as np

for _p in ("/opt/trn_rl_repo", "/root/.axon_site/_ro/trn_rl_repo"):
    if _p not in sys.path:
        sys.path.append(_p)

B = 64          # batch (x is [64, 1, 3072])
IN = 3072       # in_features
OUT = 12288     # out_features
BLOCK = 32      # quant block
NB = IN // BLOCK            # 96 blocks per row
NCORES = 8
OSH = OUT // NCORES         # 1536 out features per core
KT = IN // 128              # 24 contraction k-tiles
NL0 = 16                    # k-tiles using scale layout L0
NG = 3                      # o-groups of N=512 per core
NBC = OSH // BLOCK          # 48 bias blocks per core

# Per-k-tile type:
#   F = fp16 raw codes from host, HWDGE   (DVE mul; -128 via correction)
#   A = uint8 codes, HWDGE                (ACT cast (q-128), DVE mul)
#   C = uint8 codes, SWDGE cast-DMA->fp16 (DVE mul; -128 via correction)
# 2 F tiles give the DVE an instant start (no cast dependency); 12 A keep
# ACT at ~19us; 10 C ride the DMA engines' in-flight conversion (u8 HBM
# reads, measured-exact cast).
TYPES = ["F", "F", "A", "A", "A", "A", "C", "C", "C", "C",
         "A", "A", "A", "A", "C", "C", "C", "C",
         "A", "A", "A", "A", "C", "C"]
A_KTS = [kt for kt in range(KT) if TYPES[kt] == "A"]
F_KTS = [kt for kt in range(KT) if TYPES[kt] == "F"]
C_KTS = [kt for kt in range(KT) if TYPES[kt] == "C"]
CORR_KTS = sorted(F_KTS + C_KTS)   # tiles multiplied with RAW codes

# Code chunks in DMA issue order per ring.
HW_CHUNKS = [[0, 1], [2, 3, 4, 5], [10, 11, 12, 13], [18, 19, 20, 21]]
SW_CHUNKS = [[6, 7, 8, 9], [14, 15, 16, 17], [22, 23]]

# Compute-emission order (per-engine queues are strictly in-order; this
# roughly matches data arrival so no queue stalls on a later tile).
EMIT_ORDER = [0, 1, 2, 3, 6, 7, 4, 5, 8, 9,
              10, 11, 14, 15, 12, 13, 16, 17,
              18, 19, 22, 23, 20, 21]

_CACHE: dict = {}


def _patch_drain_split():
    """The TRN2 ISA gives every instruction exactly ONE inline wait slot;
    Tile's kernel-tail drain asks for the whole global clock (~11 sems) on a
    single instruction, which walrus sometimes refuses ("Too many sync wait
    commands").  Pre-spread those waits across one SP nop per semaphore; the
    drain's own waits then elide via the SP engine clock."""
    from concourse import tile as tile_mod

    if getattr(tile_mod.TileContext, "_drain_split_patched", False):
        return
    from concourse.vector_clock import ScopedClock, VectorClock

    orig = tile_mod.TileContext._drain_and_barrier

    def patched(self, tick_clock, wait_clock):
        gvc = tick_clock.global_clock
        n = len(gvc)
        for p in range(n):
            t = gvc[p]
            if t <= 0:
                continue
            vc = VectorClock([0] * n)
            vc.require_at_least(p, t)
            nop = self.nc.sync.nop(hint="drain_wait_split", nofuse=True)
            wait_clock.add_sem_waits(nop.ins, ScopedClock({None: vc}))
        return orig(self, tick_clock, wait_clock)

    tile_mod.TileContext._drain_and_barrier = patched
    tile_mod.TileContext._drain_split_patched = True


def _build_nc():
    import concourse.bass as bass
    import concourse.mybir as mybir
    from concourse.tile import TileContext
    from contextlib import ExitStack

    _patch_drain_split()

    f32 = mybir.dt.float32
    i32 = mybir.dt.int32
    f16 = mybir.dt.float16
    u8 = mybir.dt.uint8
    Copy = mybir.ActivationFunctionType.Copy

    nU, nF = len(A_KTS) + len(C_KTS), len(F_KTS)

    nc = bass.Bass()
    # Host-permuted/transposed codes, partition-major, grouped by type in
    # chunk order.
    wqt = nc.declare_dram_parameter("wqt", [128, nU * OSH], u8, isOutput=False)
    wqf = nc.declare_dram_parameter("wqf", [128, nF * OSH], f16, isOutput=False)
    # Scale layouts L0 | L1, each [128, 1536] fp16.
    sc = nc.declare_dram_parameter("sc", [128, 2 * OSH], f16, isOutput=False)
    # Host-permuted x^T (fp16), partition-major [128, 25*64].
    xt = nc.declare_dram_parameter("xt", [128, (KT + 1) * B], f16, isOutput=False)
    # bias bytes: [bq int32 x 1536 | bs fp16 replicated x32 -> 1536 values]
    bb = nc.declare_dram_parameter("bb", [1, 4 * OSH + 2 * OSH], u8, isOutput=False)
    y = nc.declare_dram_parameter("y", [B, OSH], f32, isOutput=True)

    # wqt (u8) holds A tiles then C tiles, in chunk order; wqf holds F.
    u8_order = [k for ch in ([c for c in HW_CHUNKS if TYPES[c[0]] == "A"] + SW_CHUNKS) for k in ch]
    u8_pos = {kt: i for i, kt in enumerate(u8_order)}
    f_pos = {kt: i for i, kt in enumerate(F_KTS)}

    with TileContext(nc) as tc, ExitStack() as ctx:
        const = ctx.enter_context(tc.tile_pool(name="const", bufs=1))
        # One buffer per tile (no reuse): each producer carries exactly ONE
        # sem wait, so no per-tile absorber ops.  (In-place DVE multiply was
        # tried: operand overlap disables the 2x_1p mode, 2.2x slower.)
        q16_pool = ctx.enter_context(tc.tile_pool(name="q16", bufs=len(A_KTS)))
        wp_pool = ctx.enter_context(tc.tile_pool(name="wp", bufs=KT))
        ysb_pool = ctx.enter_context(tc.tile_pool(name="ysb", bufs=1))
        py_pool = ctx.enter_context(tc.tile_pool(name="py", bufs=1, space="PSUM"))
        scrap_pool = ctx.enter_context(tc.tile_pool(name="scrap", bufs=1, space="PSUM"))

        # --- input DMAs --------------------------------------------------
        # SP HWDGE ring: scales-L0 first (gates every DVE multiply), then
        # the F chunk (fp16 — instant DVE head) and the A u8 chunks; y
        # rides out later (6 of the 8 lanes).
        sc_sb = const.tile([128, 2 * OSH], f16)
        nc.sync.dma_start(sc_sb[:, 0:OSH], sc[:, 0:OSH])

        chunk_tiles = {}

        def _code_dma(ci, kts, swdge):
            t = TYPES[kts[0]]
            n = len(kts)
            if t == "F":
                tile = const.tile([128, n * OSH], f16, name=f"cf{ci}")
                pos = [f_pos[k] for k in kts]
                assert pos == list(range(pos[0], pos[0] + n))
                nc.sync.dma_start(tile[:], wqf[:, pos[0] * OSH : (pos[0] + n) * OSH])
            else:
                pos = [u8_pos[k] for k in kts]
                assert pos == list(range(pos[0], pos[0] + n))
                src = wqt[:, pos[0] * OSH : (pos[0] + n) * OSH]
                if swdge:
                    # in-flight u8 -> fp16 cast in the SDMA datapath
                    tile = const.tile([128, n * OSH], f16, name=f"cc{ci}")
                    nc.gpsimd.dma_start(tile[:], src)
                else:
                    tile = const.tile([128, n * OSH], u8, name=f"cu{ci}")
                    nc.sync.dma_start(tile[:], src)
            for j, k in enumerate(kts):
                chunk_tiles[k] = tile[:, j * OSH : (j + 1) * OSH]

        # GPSIMD SWDGE ring (separate completion-sem space): xt + bias
        # bytes + scales-L1 head, then the cast chunks, interleaved with
        # the HWDGE issues so both streams start early.
        xt_sb = const.tile([128, (KT + 1) * B], f16)
        nc.gpsimd.dma_start(xt_sb[:], xt[:, :])
        bb_sb = const.tile([1, 4 * OSH + 2 * OSH], u8)
        nc.gpsimd.dma_start(bb_sb[:], bb[:, :])
        nc.gpsimd.dma_start(sc_sb[:, OSH : 2 * OSH], sc[:, OSH : 2 * OSH])
        for ci, kts in enumerate(HW_CHUNKS):
            _code_dma(ci, kts, swdge=False)
        for ci, kts in enumerate(SW_CHUNKS):
            _code_dma(10 + ci, kts, swdge=True)
        bq_sb = bb_sb[0:1, 0 : 4 * OSH].bitcast(i32)
        bsx_sb = bb_sb[0:1, 4 * OSH : 6 * OSH].bitcast(f16)

        def sc_slice(kt):
            s = 0 if kt < NL0 else 1
            return sc_sb[:, s * OSH : (s + 1) * OSH]

        # --- small prologue ----------------------------------------------
        scr_d = const.tile([1, 8], f32)
        ones1 = const.tile([1, B], f16)
        nc.vector.memset(ones1[:], 1.0)
        # Touch sc halves + bias scales once on DVE so later DVE consumers'
        # waits are engine-order-covered (then stripped).
        nc.vector.tensor_copy(scr_d[0:1, 0:1], sc_sb[0:1, 0:1])
        nc.vector.tensor_copy(scr_d[0:1, 1:2], sc_sb[0:1, OSH : OSH + 1])
        nc.vector.tensor_copy(scr_d[0:1, 2:3], bsx_sb[0:1, 0:1])
        # bias tiles; the (bq-128) ACT cast is emitted mid-loop so it does
        # not block the ACT queue's head (costed ~4us of head when first)
        bias_q16 = const.tile([1, OSH], f16)
        bias16 = const.tile([1, OSH], f16)

        # PE wait-absorber for the one-time xt DMA (matmul LW struct carries
        # at most one sync wait).
        scrap = scrap_pool.tile([1, 4], f32)
        nc.tensor.matmul(
            scrap[0:1, 0:1], xt_sb[:, 0:1], xt_sb[:, 0:1], start=True, stop=True
        )

        # --- F-tile -128 correction lhsT ---------------------------------
        # fp32 reduce of the F tiles' x slices, exact hi+lo fp16 split.
        xsum16 = {}

        def _emit_xsum():
            for sel, kts in ((0, [k for k in CORR_KTS if k < NL0]),
                             (1, [k for k in CORR_KTS if k >= NL0])):
                if not kts:
                    continue
                acc = const.tile([128, B], f32, name=f"xsumf{sel}")
                spans = []
                for k in kts:
                    if spans and k == spans[-1][1]:
                        spans[-1] = (spans[-1][0], k + 1)
                    else:
                        spans.append((k, k + 1))
                first = True
                for (a, bnd) in spans:
                    n = bnd - a
                    view = xt_sb[:, a * B : bnd * B].rearrange(
                        "p (n b) -> p b n", n=n
                    )
                    if first:
                        nc.vector.tensor_reduce(
                            acc[:], view, mybir.AxisListType.X, mybir.AluOpType.add
                        )
                        first = False
                    else:
                        part = const.tile([128, B], f32, name=f"xsp{sel}_{a}")
                        nc.vector.tensor_reduce(
                            part[:], view, mybir.AxisListType.X, mybir.AluOpType.add
                        )
                        nc.vector.tensor_add(acc[:], acc[:], part[:])
                nc.vector.tensor_scalar_mul(acc[:], acc[:], -128.0)
                hi = const.tile([128, B], f16, name=f"xsumhi{sel}")
                nc.vector.tensor_copy(hi[:], acc[:])
                res = const.tile([128, B], f32, name=f"xsumr{sel}")
                nc.vector.tensor_tensor(
                    res[:], acc[:], hi[:], mybir.AluOpType.subtract
                )
                lo = const.tile([128, B], f16, name=f"xsumlo{sel}")
                nc.vector.tensor_copy(lo[:], res[:])
                xsum16[sel] = (hi, lo)

        # --- main pipeline ------------------------------------------------
        y_sb = ysb_pool.tile([B, OSH], f32)
        py = [py_pool.tile([B, 512], f32, name=f"py{g}") for g in range(NG)]

        for ei, kt in enumerate(EMIT_ORDER):
            if TYPES[kt] == "A":
                q16 = q16_pool.tile([128, OSH], f16)
                nc.scalar.activation(q16[:], chunk_tiles[kt], Copy, bias=-128.0)
                src = q16
            else:
                src = chunk_tiles[kt]
            wp = wp_pool.tile([128, OSH], f16)
            nc.vector.tensor_mul(wp[:], src[:], sc_slice(kt))
            for g in range(NG):
                nc.tensor.matmul(
                    py[g][:],
                    xt_sb[:, B * kt : B * (kt + 1)],
                    wp[:, 512 * g : 512 * (g + 1)],
                    start=ei == 0,
                    stop=False,
                )
            if ei == 2:
                _emit_xsum()
            if ei == 8:
                nc.scalar.activation(bias_q16[:], bq_sb, Copy, bias=-128.0)
            if ei == 5:
                for sel, (hi, lo) in xsum16.items():
                    for part in (hi, lo):
                        for g in range(NG):
                            nc.tensor.matmul(
                                py[g][:],
                                part[:],
                                sc_sb[:, sel * OSH + 512 * g : sel * OSH + 512 * (g + 1)],
                                start=False,
                                stop=False,
                            )
            if ei == 18:
                # bias16 = (bq-128) * bsx, fp16 2x TT; feeds the closing mms
                nc.vector.tensor_mul(bias16[:], bias_q16[:], bsx_sb)

        # bias via K=1 matmuls against the ones row, closing accumulation;
        # evacuation split ACT/DVE/ACT so the three banks drain in ~2 serial
        # copies instead of 3.
        for g in range(NG):
            nc.tensor.matmul(
                py[g][:],
                ones1[0:1, :],
                bias16[0:1, 512 * g : 512 * (g + 1)],
                start=False,
                stop=True,
            )
        for g in range(NG):
            nc.scalar.copy(y_sb[:, 512 * g : 512 * (g + 1)], py[g][:])
        nc.sync.dma_start(y[:, :], y_sb[:])

    _strip_self_waits(nc, mybir)
    return nc


_ENGINE_SEM_PREFIX = {
    "PE": "PE_",
    "DVE": "DVE_",
    "Activation": "Activation_",
    "SP": "SP_",
}


def _strip_self_waits(nc, mybir):
    """Several TRN2 ISA instruction structs encode at most ONE sync wait
    (walrus: "Too many sync wait commands").  Two classes of Tile-emitted
    waits are redundant and safe to drop from instructions carrying >=2:

    1. Self-engine waits: an engine completes its own instructions in order.
    2. Waits already observed (same value or higher) by an EARLIER
       instruction on the same in-order engine.

    Pool (GPSIMD) is special: the 8 Q7 cores do NOT complete in a single
    program order (so Pool_ self-sem waits are load-bearing and never
    dropped), but the Pool NX sequencer still DISPATCHES in order, and sem
    waits gate dispatch: a wait on an external sem already waited for by an
    earlier Pool instruction is dispatch-covered and droppable.

    Only monotonic sems (engine clocks, DMA lanes) may be deduped: barrier
    sems are reset by sem-subtract between rounds, so a repeated wait value
    there is NOT redundant (deduping one deadlocks the kernel).
    """
    fn = nc.m.functions[0]
    observed: dict = {}
    _MONO = ("DMAHW", "DMASW", "PE_", "DVE_", "Activation_", "SP_", "Pool_")

    def _dedupable(w):
        return w.ant_name.startswith(_MONO)

    for b in fn.blocks:
        for inst in b.instructions:
            si = inst.sync_info
            if si is None or not si.on_wait:
                continue
            eng = str(inst.engine)
            if eng.split(".")[-1] == "Pool":
                keep = [
                    w
                    for w in si.on_wait
                    if w.ant_name.startswith("Pool")
                    or not _dedupable(w)
                    or observed.get((eng, w.ant_name), 0) < w.wait_value
                ]
                for w in keep:
                    if _dedupable(w) and not w.ant_name.startswith("Pool"):
                        k = (eng, w.ant_name)
                        observed[k] = max(observed.get(k, 0), w.wait_value)
                if len(keep) != len(si.on_wait):
                    inst.sync_info = mybir.SyncInfo(
                        on_wait=keep, on_update=si.on_update
                    )
                continue
            if len(si.on_wait) < 2:
                for w in si.on_wait:
                    if _dedupable(w):
                        k = (eng, w.ant_name)
                        observed[k] = max(observed.get(k, 0), w.wait_value)
                continue
            keep = [
                w
                for w in si.on_wait
                if not _dedupable(w)
                or observed.get((eng, w.ant_name), 0) < w.wait_value
            ]
            pref = _ENGINE_SEM_PREFIX.get(str(inst.engine).split(".")[-1])
            if pref is not None:
                keep = [w for w in keep if not w.ant_name.startswith(pref)]
            if len(keep) >= 2 and type(inst).__name__ == "InstDMACopy":
                if any(
                    not w.ant_name.startswith(("DMAHW", "DMASW")) for w in keep
                ):
                    keep = [
                        w
                        for w in keep
                        if not w.ant_name.startswith(("DMAHW", "DMASW"))
                    ]
            for w in keep:
                if _dedupable(w):
                    k = (eng, w.ant_name)
                    observed[k] = max(observed.get(k, 0), w.wait_value)
            if len(keep) != len(si.on_wait):
                inst.sync_info = mybir.SyncInfo(
                    on_wait=keep, on_update=si.on_update
                )


def _get_nc():
    if "nc" not in _CACHE:
        _CACHE["nc"] = _build_nc()
    return _CACHE["nc"]


def _slot_permutation():
    """slot (kt, p) -> global i = 32*block + j.  16 L0 k-tiles map lane p to
    block p mod 96 (j = kt for p<96, 16+kt else); 8 L1 k-tiles map lane p to
    block 32 + p mod 64 (j = 16+g for p<64, 24+g else).  Bijective onto
    0..3071 (each (block, j) covered exactly once)."""
    i_slot = np.empty((KT, 128), dtype=np.int64)
    p = np.arange(128)
    for kt in range(NL0):
        b = np.where(p < 96, p, p - 96)
        j = np.where(p < 96, kt, 16 + kt)
        i_slot[kt] = 32 * b + j
    for g in range(KT - NL0):
        b = 32 + (p % 64)
        j = np.where(p < 64, 16 + g, 24 + g)
        i_slot[NL0 + g] = 32 * b + j
    return i_slot


def _make_in_maps(x, w_q, w_scales, b_q, b_scales):
    i_slot = _slot_permutation()
    flat = i_slot.reshape(-1)
    p = np.arange(128)
    r0_idx = np.where(p < 96, p, p - 96)
    r1_idx = 32 + (p % 64)

    x2 = np.ascontiguousarray(x.reshape(B, IN), dtype=np.float32)
    xtp = np.zeros((IN + 128, B), dtype=np.float16)               # [3200, 64]
    xtp[:IN] = x2[:, flat].T.astype(np.float16)
    xtp[IN] = 1.0
    # partition-major: [128, 25*64], row p = slot (kt, p) over all k-tiles
    xtp = np.ascontiguousarray(
        xtp.reshape(KT + 1, 128, B).transpose(1, 0, 2).reshape(128, (KT + 1) * B)
    )

    W8 = w_q.reshape(OUT, IN).astype(np.uint8)
    W8g = W8[:, flat]                                             # [OUT, 3072]
    ws_full = np.asarray(w_scales)                                # [12288, 96]
    bq_full = np.ascontiguousarray(b_q.reshape(OUT))
    bs_full = np.ascontiguousarray(b_scales)

    in_maps = []
    for c in range(NCORES):
        o0, o1 = c * OSH, (c + 1) * OSH
        allk = np.ascontiguousarray(
            W8g[o0:o1].T.reshape(KT, 128, OSH).transpose(1, 0, 2)
        )  # [128, KT, OSH]
        u8_order = [
            k
            for ch in ([c for c in HW_CHUNKS if TYPES[c[0]] == "A"] + SW_CHUNKS)
            for k in ch
        ]
        wqt_c = np.ascontiguousarray(
            allk[:, u8_order, :].reshape(128, len(u8_order) * OSH)
        )
        wqf_c = np.ascontiguousarray(
            allk[:, F_KTS, :].astype(np.float16).reshape(128, len(F_KTS) * OSH)
        )
        ws_c = ws_full[o0:o1].astype(np.float16)                  # [1536, 96]
        L0 = ws_c[:, r0_idx].T                                    # [128, 1536]
        L1 = ws_c[:, r1_idx].T
        sc_c = np.ascontiguousarray(np.concatenate([L0, L1], axis=1))
        bsx = np.repeat(
            bs_full[o0 // BLOCK : o1 // BLOCK].astype(np.float16), BLOCK
        )                                                         # [1536] f16
        bb_c = np.frombuffer(
            bq_full[o0:o1].astype("<i4").tobytes() + bsx.tobytes(),
            dtype=np.uint8,
        ).reshape(1, 6 * OSH)
        in_maps.append(
            {
                "wqt": wqt_c,
                "wqf": wqf_c,
                "sc": sc_c,
                "xt": xtp,
                "bb": bb_c,
            }
        )
    return in_maps


def run_shards(x, w_q, w_scales, b_q, b_scales, trace=False):
    """Run the SPMD kernel; returns (y_full, BassKernelResults)."""
    from concourse.bass_utils import run_bass_kernel_spmd

    nc = _get_nc()
    in_maps = _make_in_maps(x, w_q, w_scales, b_q, b_scales)
    res = run_bass_kernel_spmd(
        nc, in_maps, core_ids=list(range(NCORES)), trace=trace
    )
    shards = [np.asarray(res.results[c]["y"]) for c in range(NCORES)]
    y = np.concatenate(shards, axis=1).reshape(B, 1, OUT)
    return y, res


def kernel(**inputs):
    y, _ = run_shards(
        inputs["x"],
        inputs["w_q"],
        inputs["w_scales"],
        inputs["b_q"],
        inputs["b_scales"],
        trace=False,
    )
    return y.astype(np.float32)


# revision 63
# speedup vs baseline: 1.0437x; 1.0437x over previous
"""DequantingLinear Trainium2 kernel — transposed-codes redesign.

y = x @ W^T + b where W = (w_q - 128) * w_scales (GGML Q8_0-style, block=32),
b = (b_q - 128) * b_scales.

Sharding: column-parallel over out_features across 8 cores (1536 rows of W
per core).  Design vs the first-generation kernel (~89 us):

1. The int32 codes carry one useful byte; the host repacks them (pure
   storage change, values identical): uint8 for half the k-tiles, float16
   (still the raw 0..255 code values) for the other half — the dominant
   HBM stream shrinks ~3x.
2. The host pre-TRANSPOSES the code matrix to [in, out] layout (layout
   only, like the x transpose), so the PE consumes dequantized tiles
   directly: no PE transposes, no PSUM evacuation traffic.  PE work per
   core collapses to 24 k-tiles x 3 N=512 matmuls + a few extras.
3. Block-to-partition permutation: a k-tile of 128 i-rows normally spans 4
   quant blocks, making the scale operand a cross-partition gather.  We
   instead permute which i lands on which (k-tile, partition) slot so each
   lane's scale is constant per tile and the scale operand is an ordinary
   [128, 1536] step-1 fp16 tile: 16 "L0" k-tiles (lane p -> block p mod 96)
   and 8 "L1" k-tiles (lane p -> block 32 + p mod 64), covering each
   (block, j) exactly once.  x is permuted identically on the host.
4. Dequant engine split (HW-measured: DVE tensor_tensor 2x_1p 938ns/tile,
   ACT cast 1.56us, DVE STT 1x 1.74us; GPSIMD elementwise shares SBUF
   ports with DVE — running it measured DVE TTs at 3.7us — so GPSIMD only
   triggers SWDGE DMAs here):
   - "A" tiles (uint8): ACT activation(Copy, bias=-128) -> fp16, then DVE
     2x tensor_mul by the scale tile.
   - "F" tiles (float16 raw codes): DVE 2x tensor_mul directly; the
     missing -128 shift is restored by 3 matmuls per scale layout with
     lhsT = -128 * sum of those tiles' x slices (matmul is linear in lhsT
     and the rhs — the scale tile — is shared).  The sum is reduced in
     fp32 and split into exact hi+lo fp16 parts (two matmul sets).
5. Bias: ACT casts (bq-128)->fp16 (otherwise idle), DVE multiplies by the
   host-REPLICATED per-block scales (plain 1-D APs; a (1,48,32) sub-dim AP
   measured ~3x slower), added via K=1 matmuls against a ones row.
6. DMA choreography: the SP HWDGE ring carries [scales-L0, then u8/f16
   code chunks interleaved, y] — exactly its 8 completion lanes; GPSIMD
   SWDGE (own lane space) carries xt, bias bytes, and scales-L1 (needed
   late).  scales-L0 first + interleaved chunk types keep ACT (casts) and
   DVE (multiplies) fed from ~8us; partition-major DRAM layouts keep every
   transfer at 128 large descriptors (a [3200,64] rearranged xt DMA
   measured 15.6us of descriptor generation; partition-major is ~1us).

Two TRN2 toolchain quirks are handled explicitly (see _strip_self_waits
and _patch_drain_split): several instruction structs encode at most ONE
semaphore wait (walrus "Too many sync wait commands"), and the kernel-tail
drain's global-clock waits are pre-spread across SP nops.  Producers are
arranged to carry exactly one wait (one-buffer-per-tile pools, head
absorbers for the scale/bias tiles); a post-pass drops provably redundant
waits.  Barrier semaphores are reset between rounds, so the post-pass only
dedupes waits on monotonic sems (engine clocks + DMA lanes) — deduping a
barrier wait deadlocks the kernel (found the hard way).
"""

import sys

import numpy as np

for _p in ("/opt/trn_rl_repo", "/root/.axon_site/_ro/trn_rl_repo"):
    if _p not in sys.path:
        sys.path.append(_p)

B = 64          # batch (x is [64, 1, 3072])
IN = 3072       # in_features
OUT = 12288     # out_features
BLOCK = 32      # quant block
NB = IN // BLOCK            # 96 blocks per row
NCORES = 8
OSH = OUT // NCORES         # 1536 out features per core
KT = IN // 128              # 24 contraction k-tiles
NL0 = 16                    # k-tiles using scale layout L0
NG = 3                      # o-groups of N=512 per core
NBC = OSH // BLOCK          # 48 bias blocks per core

# Per-k-tile type:
#   F = fp16 raw codes from host, HWDGE   (DVE mul; -128 via correction)
#   A = uint8 codes, HWDGE                (ACT cast (q-128), DVE mul)
#   C = uint8 codes, SWDGE cast-DMA->fp16 (DVE mul; -128 via correction)
# 2 F tiles give the DVE an instant start (no cast dependency); 12 A keep
# ACT at ~19us; 10 C ride the DMA engines' in-flight conversion (u8 HBM
# reads, measured-exact cast).
TYPES = (["A"] * 4 + ["F"] * 4) * 3
A_KTS = [kt for kt in range(KT) if TYPES[kt] == "A"]
F_KTS = [kt for kt in range(KT) if TYPES[kt] == "F"]
C_KTS = [kt for kt in range(KT) if TYPES[kt] == "C"]
CORR_KTS = sorted(F_KTS + C_KTS)   # tiles multiplied with RAW codes

# Code chunks in DMA issue order per ring (SWDGE cast-chunks measured a
# net loss vs HWDGE u8+ACT-cast — SW_CHUNKS kept empty).
HW_CHUNKS = [
    [0, 1, 2, 3],          # u8
    [4, 5, 6, 7],          # f16
    [8, 9, 10, 11],        # u8
    [12, 13, 14, 15],      # f16
    [16, 17, 18, 19],      # u8
    [20, 21, 22, 23],      # f16
]
SW_CHUNKS = []

# Compute-emission order (per-engine queues are strictly in-order; this
# roughly matches data arrival so no queue stalls on a later tile).
EMIT_ORDER = [0, 1, 2, 3, 4, 5, 6, 7,
              8, 9, 12, 13, 10, 14, 11, 15,
              16, 17, 20, 21, 18, 22, 19, 23]

_CACHE: dict = {}


def _patch_drain_split():
    """The TRN2 ISA gives every instruction exactly ONE inline wait slot;
    Tile's kernel-tail drain asks for the whole global clock (~11 sems) on a
    single instruction, which walrus sometimes refuses ("Too many sync wait
    commands").  Pre-spread those waits across one SP nop per semaphore; the
    drain's own waits then elide via the SP engine clock."""
    from concourse import tile as tile_mod

    if getattr(tile_mod.TileContext, "_drain_split_patched", False):
        return
    from concourse.vector_clock import ScopedClock, VectorClock

    orig = tile_mod.TileContext._drain_and_barrier

    def patched(self, tick_clock, wait_clock):
        gvc = tick_clock.global_clock
        n = len(gvc)
        for p in range(n):
            t = gvc[p]
            if t <= 0:
                continue
            vc = VectorClock([0] * n)
            vc.require_at_least(p, t)
            nop = self.nc.sync.nop(hint="drain_wait_split", nofuse=True)
            wait_clock.add_sem_waits(nop.ins, ScopedClock({None: vc}))
        return orig(self, tick_clock, wait_clock)

    tile_mod.TileContext._drain_and_barrier = patched
    tile_mod.TileContext._drain_split_patched = True


def _build_nc():
    import concourse.bass as bass
    import concourse.mybir as mybir
    from concourse.tile import TileContext
    from contextlib import ExitStack

    _patch_drain_split()

    f32 = mybir.dt.float32
    i32 = mybir.dt.int32
    f16 = mybir.dt.float16
    u8 = mybir.dt.uint8
    Copy = mybir.ActivationFunctionType.Copy

    nU, nF = len(A_KTS) + len(C_KTS), len(F_KTS)

    nc = bass.Bass()
    # Host-permuted/transposed codes, partition-major, grouped by type in
    # chunk order.
    wqt = nc.declare_dram_parameter("wqt", [128, nU * OSH], u8, isOutput=False)
    wqf = nc.declare_dram_parameter("wqf", [128, nF * OSH], f16, isOutput=False)
    # Scale layouts L0 | L1, each [128, 1536] fp16.
    sc = nc.declare_dram_parameter("sc", [128, 2 * OSH], f16, isOutput=False)
    # Host-permuted x^T (fp16), partition-major [128, 25*64].
    xt = nc.declare_dram_parameter("xt", [128, (KT + 1) * B], f16, isOutput=False)
    # bias bytes: [bq int32 x 1536 | bs fp16 replicated x32 -> 1536 values]
    bb = nc.declare_dram_parameter("bb", [1, 4 * OSH + 2 * OSH], u8, isOutput=False)
    y = nc.declare_dram_parameter("y", [B, OSH], f32, isOutput=True)

    # wqt (u8) holds A tiles then C tiles, in chunk order; wqf holds F.
    u8_order = [k for ch in ([c for c in HW_CHUNKS if TYPES[c[0]] == "A"] + SW_CHUNKS) for k in ch]
    u8_pos = {kt: i for i, kt in enumerate(u8_order)}
    f_pos = {kt: i for i, kt in enumerate(F_KTS)}

    with TileContext(nc) as tc, ExitStack() as ctx:
        const = ctx.enter_context(tc.tile_pool(name="const", bufs=1))
        # One buffer per tile (no reuse): each producer carries exactly ONE
        # sem wait, so no per-tile absorber ops.  (In-place DVE multiply was
        # tried: operand overlap disables the 2x_1p mode, 2.2x slower.)
        q16_pool = ctx.enter_context(tc.tile_pool(name="q16", bufs=len(A_KTS)))
        wp_pool = ctx.enter_context(tc.tile_pool(name="wp", bufs=KT))
        ysb_pool = ctx.enter_context(tc.tile_pool(name="ysb", bufs=1))
        py_pool = ctx.enter_context(tc.tile_pool(name="py", bufs=1, space="PSUM"))
        scrap_pool = ctx.enter_context(tc.tile_pool(name="scrap", bufs=1, space="PSUM"))

        # --- input DMAs --------------------------------------------------
        # SP HWDGE ring: scales-L0 first (gates every DVE multiply), then
        # the F chunk (fp16 — instant DVE head) and the A u8 chunks; y
        # rides out later (6 of the 8 lanes).
        sc_sb = const.tile([128, 2 * OSH], f16)
        nc.sync.dma_start(sc_sb[:, 0:OSH], sc[:, 0:OSH])

        chunk_tiles = {}

        def _code_dma(ci, kts, swdge):
            t = TYPES[kts[0]]
            n = len(kts)
            if t == "F":
                tile = const.tile([128, n * OSH], f16, name=f"cf{ci}")
                pos = [f_pos[k] for k in kts]
                assert pos == list(range(pos[0], pos[0] + n))
                nc.sync.dma_start(tile[:], wqf[:, pos[0] * OSH : (pos[0] + n) * OSH])
            else:
                pos = [u8_pos[k] for k in kts]
                assert pos == list(range(pos[0], pos[0] + n))
                src = wqt[:, pos[0] * OSH : (pos[0] + n) * OSH]
                if swdge:
                    # in-flight u8 -> fp16 cast in the SDMA datapath
                    tile = const.tile([128, n * OSH], f16, name=f"cc{ci}")
                    nc.gpsimd.dma_start(tile[:], src)
                else:
                    tile = const.tile([128, n * OSH], u8, name=f"cu{ci}")
                    nc.sync.dma_start(tile[:], src)
            for j, k in enumerate(kts):
                chunk_tiles[k] = tile[:, j * OSH : (j + 1) * OSH]

        # GPSIMD SWDGE ring (separate completion-sem space): xt + bias
        # bytes + scales-L1 head, then the cast chunks, interleaved with
        # the HWDGE issues so both streams start early.
        xt_sb = const.tile([128, (KT + 1) * B], f16)
        nc.gpsimd.dma_start(xt_sb[:], xt[:, :])
        bb_sb = const.tile([1, 4 * OSH + 2 * OSH], u8)
        nc.gpsimd.dma_start(bb_sb[:], bb[:, :])
        nc.gpsimd.dma_start(sc_sb[:, OSH : 2 * OSH], sc[:, OSH : 2 * OSH])
        for ci, kts in enumerate(HW_CHUNKS):
            _code_dma(ci, kts, swdge=False)
        for ci, kts in enumerate(SW_CHUNKS):
            _code_dma(10 + ci, kts, swdge=True)
        bq_sb = bb_sb[0:1, 0 : 4 * OSH].bitcast(i32)
        bsx_sb = bb_sb[0:1, 4 * OSH : 6 * OSH].bitcast(f16)

        def sc_slice(kt):
            s = 0 if kt < NL0 else 1
            return sc_sb[:, s * OSH : (s + 1) * OSH]

        # --- small prologue ----------------------------------------------
        scr_d = const.tile([1, 8], f32)
        ones1 = const.tile([1, B], f16)
        nc.vector.memset(ones1[:], 1.0)
        # Touch sc halves + bias scales once on DVE so later DVE consumers'
        # waits are engine-order-covered (then stripped).
        nc.vector.tensor_copy(scr_d[0:1, 0:1], sc_sb[0:1, 0:1])
        nc.vector.tensor_copy(scr_d[0:1, 1:2], sc_sb[0:1, OSH : OSH + 1])
        nc.vector.tensor_copy(scr_d[0:1, 2:3], bsx_sb[0:1, 0:1])
        # bias tiles; the (bq-128) ACT cast is emitted mid-loop so it does
        # not block the ACT queue's head (costed ~4us of head when first)
        bias_q16 = const.tile([1, OSH], f16)
        bias16 = const.tile([1, OSH], f16)

        # PE wait-absorber for the one-time xt DMA (matmul LW struct carries
        # at most one sync wait).
        scrap = scrap_pool.tile([1, 4], f32)
        nc.tensor.matmul(
            scrap[0:1, 0:1], xt_sb[:, 0:1], xt_sb[:, 0:1], start=True, stop=True
        )

        # --- F-tile -128 correction lhsT ---------------------------------
        # fp32 reduce of the F tiles' x slices, exact hi+lo fp16 split.
        xsum16 = {}

        def _emit_xsum():
            for sel, kts in ((0, [k for k in CORR_KTS if k < NL0]),
                             (1, [k for k in CORR_KTS if k >= NL0])):
                if not kts:
                    continue
                acc = const.tile([128, B], f32, name=f"xsumf{sel}")
                spans = []
                for k in kts:
                    if spans and k == spans[-1][1]:
                        spans[-1] = (spans[-1][0], k + 1)
                    else:
                        spans.append((k, k + 1))
                first = True
                for (a, bnd) in spans:
                    n = bnd - a
                    view = xt_sb[:, a * B : bnd * B].rearrange(
                        "p (n b) -> p b n", n=n
                    )
                    if first:
                        nc.vector.tensor_reduce(
                            acc[:], view, mybir.AxisListType.X, mybir.AluOpType.add
                        )
                        first = False
                    else:
                        part = const.tile([128, B], f32, name=f"xsp{sel}_{a}")
                        nc.vector.tensor_reduce(
                            part[:], view, mybir.AxisListType.X, mybir.AluOpType.add
                        )
                        nc.vector.tensor_add(acc[:], acc[:], part[:])
                nc.vector.tensor_scalar_mul(acc[:], acc[:], -128.0)
                hi = const.tile([128, B], f16, name=f"xsumhi{sel}")
                nc.vector.tensor_copy(hi[:], acc[:])
                res = const.tile([128, B], f32, name=f"xsumr{sel}")
                nc.vector.tensor_tensor(
                    res[:], acc[:], hi[:], mybir.AluOpType.subtract
                )
                lo = const.tile([128, B], f16, name=f"xsumlo{sel}")
                nc.vector.tensor_copy(lo[:], res[:])
                xsum16[sel] = (hi, lo)

        # --- main pipeline ------------------------------------------------
        y_sb = ysb_pool.tile([B, OSH], f32)
        py = [py_pool.tile([B, 512], f32, name=f"py{g}") for g in range(NG)]

        for ei, kt in enumerate(EMIT_ORDER):
            if TYPES[kt] == "A":
                q16 = q16_pool.tile([128, OSH], f16)
                nc.scalar.activation(q16[:], chunk_tiles[kt], Copy, bias=-128.0)
                src = q16
            else:
                src = chunk_tiles[kt]
            wp = wp_pool.tile([128, OSH], f16)
            nc.vector.tensor_mul(wp[:], src[:], sc_slice(kt))
            for g in range(NG):
                nc.tensor.matmul(
                    py[g][:],
                    xt_sb[:, B * kt : B * (kt + 1)],
                    wp[:, 512 * g : 512 * (g + 1)],
                    start=ei == 0,
                    stop=False,
                )
            if ei == 2:
                _emit_xsum()
            if ei == 8:
                nc.scalar.activation(bias_q16[:], bq_sb, Copy, bias=-128.0)
            if ei == 5:
                for sel, (hi, lo) in xsum16.items():
                    for part in (hi, lo):
                        for g in range(NG):
                            nc.tensor.matmul(
                                py[g][:],
                                part[:],
                                sc_sb[:, sel * OSH + 512 * g : sel * OSH + 512 * (g + 1)],
                                start=False,
                                stop=False,
                            )
            if ei == 18:
                # bias16 = (bq-128) * bsx, fp16 2x TT; feeds the closing mms
                nc.vector.tensor_mul(bias16[:], bias_q16[:], bsx_sb)

        # bias via K=1 matmuls against the ones row, closing accumulation;
        # evacuation split ACT/DVE/ACT so the three banks drain in ~2 serial
        # copies instead of 3.
        for g in range(NG):
            nc.tensor.matmul(
                py[g][:],
                ones1[0:1, :],
                bias16[0:1, 512 * g : 512 * (g + 1)],
                start=False,
                stop=True,
            )
        for g in range(NG):
            nc.scalar.copy(y_sb[:, 512 * g : 512 * (g + 1)], py[g][:])
        nc.sync.dma_start(y[:, :], y_sb[:])

    _strip_self_waits(nc, mybir)
    return nc


_ENGINE_SEM_PREFIX = {
    "PE": "PE_",
    "DVE": "DVE_",
    "Activation": "Activation_",
    "SP": "SP_",
}


def _strip_self_waits(nc, mybir):
    """Several TRN2 ISA instruction structs encode at most ONE sync wait
    (walrus: "Too many sync wait commands").  Two classes of Tile-emitted
    waits are redundant and safe to drop from instructions carrying >=2:

    1. Self-engine waits: an engine completes its own instructions in order.
    2. Waits already observed (same value or higher) by an EARLIER
       instruction on the same in-order engine.

    Pool (GPSIMD) is special: the 8 Q7 cores do NOT complete in a single
    program order (so Pool_ self-sem waits are load-bearing and never
    dropped), but the Pool NX sequencer still DISPATCHES in order, and sem
    waits gate dispatch: a wait on an external sem already waited for by an
    earlier Pool instruction is dispatch-covered and droppable.

    Only monotonic sems (engine clocks, DMA lanes) may be deduped: barrier
    sems are reset by sem-subtract between rounds, so a repeated wait value
    there is NOT redundant (deduping one deadlocks the kernel).
    """
    fn = nc.m.functions[0]
    observed: dict = {}
    _MONO = ("DMAHW", "DMASW", "PE_", "DVE_", "Activation_", "SP_", "Pool_")

    def _dedupable(w):
        return w.ant_name.startswith(_MONO)

    for b in fn.blocks:
        for inst in b.instructions:
            si = inst.sync_info
            if si is None or not si.on_wait:
                continue
            eng = str(inst.engine)
            if eng.split(".")[-1] == "Pool":
                keep = [
                    w
                    for w in si.on_wait
                    if w.ant_name.startswith("Pool")
                    or not _dedupable(w)
                    or observed.get((eng, w.ant_name), 0) < w.wait_value
                ]
                for w in keep:
                    if _dedupable(w) and not w.ant_name.startswith("Pool"):
                        k = (eng, w.ant_name)
                        observed[k] = max(observed.get(k, 0), w.wait_value)
                if len(keep) != len(si.on_wait):
                    inst.sync_info = mybir.SyncInfo(
                        on_wait=keep, on_update=si.on_update
                    )
                continue
            if len(si.on_wait) < 2:
                for w in si.on_wait:
                    if _dedupable(w):
                        k = (eng, w.ant_name)
                        observed[k] = max(observed.get(k, 0), w.wait_value)
                continue
            keep = [
                w
                for w in si.on_wait
                if not _dedupable(w)
                or observed.get((eng, w.ant_name), 0) < w.wait_value
            ]
            pref = _ENGINE_SEM_PREFIX.get(str(inst.engine).split(".")[-1])
            if pref is not None:
                keep = [w for w in keep if not w.ant_name.startswith(pref)]
            if len(keep) >= 2 and type(inst).__name__ == "InstDMACopy":
                if any(
                    not w.ant_name.startswith(("DMAHW", "DMASW")) for w in keep
                ):
                    keep = [
                        w
                        for w in keep
                        if not w.ant_name.startswith(("DMAHW", "DMASW"))
                    ]
            for w in keep:
                if _dedupable(w):
                    k = (eng, w.ant_name)
                    observed[k] = max(observed.get(k, 0), w.wait_value)
            if len(keep) != len(si.on_wait):
                inst.sync_info = mybir.SyncInfo(
                    on_wait=keep, on_update=si.on_update
                )


def _get_nc():
    if "nc" not in _CACHE:
        _CACHE["nc"] = _build_nc()
    return _CACHE["nc"]


def _slot_permutation():
    """slot (kt, p) -> global i = 32*block + j.  16 L0 k-tiles map lane p to
    block p mod 96 (j = kt for p<96, 16+kt else); 8 L1 k-tiles map lane p to
    block 32 + p mod 64 (j = 16+g for p<64, 24+g else).  Bijective onto
    0..3071 (each (block, j) covered exactly once)."""
    i_slot = np.empty((KT, 128), dtype=np.int64)
    p = np.arange(128)
    for kt in range(NL0):
        b = np.where(p < 96, p, p - 96)
        j = np.where(p < 96, kt, 16 + kt)
        i_slot[kt] = 32 * b + j
    for g in range(KT - NL0):
        b = 32 + (p % 64)
        j = np.where(p < 64, 16 + g, 24 + g)
        i_slot[NL0 + g] = 32 * b + j
    return i_slot


def _make_in_maps(x, w_q, w_scales, b_q, b_scales):
    i_slot = _slot_permutation()
    flat = i_slot.reshape(-1)
    p = np.arange(128)
    r0_idx = np.where(p < 96, p, p - 96)
    r1_idx = 32 + (p % 64)

    x2 = np.ascontiguousarray(x.reshape(B, IN), dtype=np.float32)
    xtp = np.zeros((IN + 128, B), dtype=np.float16)               # [3200, 64]
    xtp[:IN] = x2[:, flat].T.astype(np.float16)
    xtp[IN] = 1.0
    # partition-major: [128, 25*64], row p = slot (kt, p) over all k-tiles
    xtp = np.ascontiguousarray(
        xtp.reshape(KT + 1, 128, B).transpose(1, 0, 2).reshape(128, (KT + 1) * B)
    )

    W8 = w_q.reshape(OUT, IN).astype(np.uint8)
    W8g = W8[:, flat]                                             # [OUT, 3072]
    ws_full = np.asarray(w_scales)                                # [12288, 96]
    bq_full = np.ascontiguousarray(b_q.reshape(OUT))
    bs_full = np.ascontiguousarray(b_scales)

    in_maps = []
    for c in range(NCORES):
        o0, o1 = c * OSH, (c + 1) * OSH
        allk = np.ascontiguousarray(
            W8g[o0:o1].T.reshape(KT, 128, OSH).transpose(1, 0, 2)
        )  # [128, KT, OSH]
        u8_order = [
            k
            for ch in ([c for c in HW_CHUNKS if TYPES[c[0]] == "A"] + SW_CHUNKS)
            for k in ch
        ]
        wqt_c = np.ascontiguousarray(
            allk[:, u8_order, :].reshape(128, len(u8_order) * OSH)
        )
        wqf_c = np.ascontiguousarray(
            allk[:, F_KTS, :].astype(np.float16).reshape(128, len(F_KTS) * OSH)
        )
        ws_c = ws_full[o0:o1].astype(np.float16)                  # [1536, 96]
        L0 = ws_c[:, r0_idx].T                                    # [128, 1536]
        L1 = ws_c[:, r1_idx].T
        sc_c = np.ascontiguousarray(np.concatenate([L0, L1], axis=1))
        bsx = np.repeat(
            bs_full[o0 // BLOCK : o1 // BLOCK].astype(np.float16), BLOCK
        )                                                         # [1536] f16
        bb_c = np.frombuffer(
            bq_full[o0:o1].astype("<i4").tobytes() + bsx.tobytes(),
            dtype=np.uint8,
        ).reshape(1, 6 * OSH)
        in_maps.append(
            {
                "wqt": wqt_c,
                "wqf": wqf_c,
                "sc": sc_c,
                "xt": xtp,
                "bb": bb_c,
            }
        )
    return in_maps


def run_shards(x, w_q, w_scales, b_q, b_scales, trace=False):
    """Run the SPMD kernel; returns (y_full, BassKernelResults)."""
    from concourse.bass_utils import run_bass_kernel_spmd

    nc = _get_nc()
    in_maps = _make_in_maps(x, w_q, w_scales, b_q, b_scales)
    res = run_bass_kernel_spmd(
        nc, in_maps, core_ids=list(range(NCORES)), trace=trace
    )
    shards = [np.asarray(res.results[c]["y"]) for c in range(NCORES)]
    y = np.concatenate(shards, axis=1).reshape(B, 1, OUT)
    return y, res


def kernel(**inputs):
    y, _ = run_shards(
        inputs["x"],
        inputs["w_q"],
        inputs["w_scales"],
        inputs["b_q"],
        inputs["b_scales"],
        trace=False,
    )
    return y.astype(np.float32)


# revision 65
# speedup vs baseline: 1.0719x; 1.0270x over previous
"""DequantingLinear Trainium2 kernel — transposed-codes redesign.

y = x @ W^T + b where W = (w_q - 128) * w_scales (GGML Q8_0-style, block=32),
b = (b_q - 128) * b_scales.

Sharding: column-parallel over out_features across 8 cores (1536 rows of W
per core).  Design vs the first-generation kernel (~89 us):

1. The int32 codes carry one useful byte; the host repacks them (pure
   storage change, values identical): uint8 for half the k-tiles, float16
   (still the raw 0..255 code values) for the other half — the dominant
   HBM stream shrinks ~3x.
2. The host pre-TRANSPOSES the code matrix to [in, out] layout (layout
   only, like the x transpose), so the PE consumes dequantized tiles
   directly: no PE transposes, no PSUM evacuation traffic.  PE work per
   core collapses to 24 k-tiles x 3 N=512 matmuls + a few extras.
3. Block-to-partition permutation: a k-tile of 128 i-rows normally spans 4
   quant blocks, making the scale operand a cross-partition gather.  We
   instead permute which i lands on which (k-tile, partition) slot so each
   lane's scale is constant per tile and the scale operand is an ordinary
   [128, 1536] step-1 fp16 tile: 16 "L0" k-tiles (lane p -> block p mod 96)
   and 8 "L1" k-tiles (lane p -> block 32 + p mod 64), covering each
   (block, j) exactly once.  x is permuted identically on the host.
4. Dequant engine split (HW-measured: DVE tensor_tensor 2x_1p 938ns/tile,
   ACT cast 1.56us, DVE STT 1x 1.74us; GPSIMD elementwise shares SBUF
   ports with DVE — running it measured DVE TTs at 3.7us — so GPSIMD only
   triggers SWDGE DMAs here):
   - "A" tiles (uint8): ACT activation(Copy, bias=-128) -> fp16, then DVE
     2x tensor_mul by the scale tile.
   - "F" tiles (float16 raw codes): DVE 2x tensor_mul directly; the
     missing -128 shift is restored by 3 matmuls per scale layout with
     lhsT = -128 * sum of those tiles' x slices (matmul is linear in lhsT
     and the rhs — the scale tile — is shared).  The sum is reduced in
     fp32 and split into exact hi+lo fp16 parts (two matmul sets).
5. Bias: ACT casts (bq-128)->fp16 (otherwise idle), DVE multiplies by the
   host-REPLICATED per-block scales (plain 1-D APs; a (1,48,32) sub-dim AP
   measured ~3x slower), added via K=1 matmuls against a ones row.
6. DMA choreography: the SP HWDGE ring carries [scales-L0, then u8/f16
   code chunks interleaved, y] — exactly its 8 completion lanes; GPSIMD
   SWDGE (own lane space) carries xt, bias bytes, and scales-L1 (needed
   late).  scales-L0 first + interleaved chunk types keep ACT (casts) and
   DVE (multiplies) fed from ~8us; partition-major DRAM layouts keep every
   transfer at 128 large descriptors (a [3200,64] rearranged xt DMA
   measured 15.6us of descriptor generation; partition-major is ~1us).

Two TRN2 toolchain quirks are handled explicitly (see _strip_self_waits
and _patch_drain_split): several instruction structs encode at most ONE
semaphore wait (walrus "Too many sync wait commands"), and the kernel-tail
drain's global-clock waits are pre-spread across SP nops.  Producers are
arranged to carry exactly one wait (one-buffer-per-tile pools, head
absorbers for the scale/bias tiles); a post-pass drops provably redundant
waits.  Barrier semaphores are reset between rounds, so the post-pass only
dedupes waits on monotonic sems (engine clocks + DMA lanes) — deduping a
barrier wait deadlocks the kernel (found the hard way).
"""

import sys

import numpy as np

for _p in ("/opt/trn_rl_repo", "/root/.axon_site/_ro/trn_rl_repo"):
    if _p not in sys.path:
        sys.path.append(_p)

B = 64          # batch (x is [64, 1, 3072])
IN = 3072       # in_features
OUT = 12288     # out_features
BLOCK = 32      # quant block
NB = IN // BLOCK            # 96 blocks per row
NCORES = 8
OSH = OUT // NCORES         # 1536 out features per core
KT = IN // 128              # 24 contraction k-tiles
NL0 = 16                    # k-tiles using scale layout L0
NG = 3                      # o-groups of N=512 per core
NBC = OSH // BLOCK          # 48 bias blocks per core

# Per-k-tile type:
#   F = fp16 raw codes from host, HWDGE   (DVE mul; -128 via correction)
#   A = uint8 codes, HWDGE                (ACT cast (q-128), DVE mul)
#   C = uint8 codes, SWDGE cast-DMA->fp16 (DVE mul; -128 via correction)
# 2 F tiles give the DVE an instant start (no cast dependency); 12 A keep
# ACT at ~19us; 10 C ride the DMA engines' in-flight conversion (u8 HBM
# reads, measured-exact cast).
TYPES = (["A"] * 4 + ["F"] * 4) * 3
A_KTS = [kt for kt in range(KT) if TYPES[kt] == "A"]
F_KTS = [kt for kt in range(KT) if TYPES[kt] == "F"]
C_KTS = [kt for kt in range(KT) if TYPES[kt] == "C"]
CORR_KTS = sorted(F_KTS + C_KTS)   # tiles multiplied with RAW codes

# Code chunks in DMA issue order per ring (SWDGE cast-chunks measured a
# net loss vs HWDGE u8+ACT-cast — SW_CHUNKS kept empty).
HW_CHUNKS = [
    [0, 1, 2, 3],          # u8
    [4, 5, 6, 7],          # f16
    [8, 9, 10, 11],        # u8
    [12, 13, 14, 15],      # f16
    [16, 17, 18, 19],      # u8
    [20, 21, 22, 23],      # f16
]
SW_CHUNKS = []

# Compute-emission order (per-engine queues are strictly in-order; this
# roughly matches data arrival so no queue stalls on a later tile).
EMIT_ORDER = [0, 1, 2, 3, 4, 5, 6, 7,
              8, 9, 12, 13, 10, 14, 11, 15,
              16, 17, 20, 21, 18, 22, 19, 23]

_CACHE: dict = {}


def _patch_drain_split():
    """The TRN2 ISA gives every instruction exactly ONE inline wait slot;
    Tile's kernel-tail drain asks for the whole global clock (~11 sems) on a
    single instruction, which walrus sometimes refuses ("Too many sync wait
    commands").  Pre-spread those waits across one SP nop per semaphore; the
    drain's own waits then elide via the SP engine clock."""
    from concourse import tile as tile_mod

    if getattr(tile_mod.TileContext, "_drain_split_patched", False):
        return
    from concourse.vector_clock import ScopedClock, VectorClock

    orig = tile_mod.TileContext._drain_and_barrier

    def patched(self, tick_clock, wait_clock):
        gvc = tick_clock.global_clock
        n = len(gvc)
        for p in range(n):
            t = gvc[p]
            if t <= 0:
                continue
            vc = VectorClock([0] * n)
            vc.require_at_least(p, t)
            nop = self.nc.sync.nop(hint="drain_wait_split", nofuse=True)
            wait_clock.add_sem_waits(nop.ins, ScopedClock({None: vc}))
        return orig(self, tick_clock, wait_clock)

    tile_mod.TileContext._drain_and_barrier = patched
    tile_mod.TileContext._drain_split_patched = True


def _build_nc():
    import concourse.bass as bass
    import concourse.mybir as mybir
    from concourse.tile import TileContext
    from contextlib import ExitStack

    _patch_drain_split()

    f32 = mybir.dt.float32
    i32 = mybir.dt.int32
    f16 = mybir.dt.float16
    u8 = mybir.dt.uint8
    Copy = mybir.ActivationFunctionType.Copy

    nU, nF = len(A_KTS) + len(C_KTS), len(F_KTS)

    nc = bass.Bass()
    # Host-permuted/transposed codes, partition-major, grouped by type in
    # chunk order.
    wqt = nc.declare_dram_parameter("wqt", [128, nU * OSH], u8, isOutput=False)
    wqf = nc.declare_dram_parameter("wqf", [128, nF * OSH], f16, isOutput=False)
    # Scale layouts L0 | L1, each [128, 1536] fp16.
    sc = nc.declare_dram_parameter("sc", [128, 2 * OSH], f16, isOutput=False)
    # Host-permuted x^T (fp16), partition-major [128, 25*64].
    xt = nc.declare_dram_parameter("xt", [128, (KT + 1) * B], f16, isOutput=False)
    # bias bytes: [bq int32 x 1536 | bs fp16 replicated x32 -> 1536 values]
    bb = nc.declare_dram_parameter("bb", [1, 4 * OSH + 2 * OSH], u8, isOutput=False)
    y = nc.declare_dram_parameter("y", [B, OSH], f32, isOutput=True)

    # wqt (u8) holds A tiles then C tiles, in chunk order; wqf holds F.
    u8_order = [k for ch in ([c for c in HW_CHUNKS if TYPES[c[0]] == "A"] + SW_CHUNKS) for k in ch]
    u8_pos = {kt: i for i, kt in enumerate(u8_order)}
    f_pos = {kt: i for i, kt in enumerate(F_KTS)}

    with TileContext(nc) as tc, ExitStack() as ctx:
        const = ctx.enter_context(tc.tile_pool(name="const", bufs=1))
        # One buffer per tile (no reuse): each producer carries exactly ONE
        # sem wait, so no per-tile absorber ops.  (In-place DVE multiply was
        # tried: operand overlap disables the 2x_1p mode, 2.2x slower.)
        q16_pool = ctx.enter_context(tc.tile_pool(name="q16", bufs=len(A_KTS)))
        wp_pool = ctx.enter_context(tc.tile_pool(name="wp", bufs=KT))
        ysb_pool = ctx.enter_context(tc.tile_pool(name="ysb", bufs=1))
        py_pool = ctx.enter_context(tc.tile_pool(name="py", bufs=1, space="PSUM"))
        scrap_pool = ctx.enter_context(tc.tile_pool(name="scrap", bufs=1, space="PSUM"))

        # --- input DMAs --------------------------------------------------
        # SP HWDGE ring: scales-L0 first (gates every DVE multiply), then
        # the F chunk (fp16 — instant DVE head) and the A u8 chunks; y
        # rides out later (6 of the 8 lanes).
        sc_sb = const.tile([128, 2 * OSH], f16)
        chunk_tiles = {}

        def _code_dma(ci, kts, swdge):
            t = TYPES[kts[0]]
            n = len(kts)
            if t == "F":
                tile = const.tile([128, n * OSH], f16, name=f"cf{ci}")
                pos = [f_pos[k] for k in kts]
                assert pos == list(range(pos[0], pos[0] + n))
                nc.sync.dma_start(tile[:], wqf[:, pos[0] * OSH : (pos[0] + n) * OSH])
            else:
                pos = [u8_pos[k] for k in kts]
                assert pos == list(range(pos[0], pos[0] + n))
                src = wqt[:, pos[0] * OSH : (pos[0] + n) * OSH]
                if swdge:
                    # in-flight u8 -> fp16 cast in the SDMA datapath
                    tile = const.tile([128, n * OSH], f16, name=f"cc{ci}")
                    nc.gpsimd.dma_start(tile[:], src)
                else:
                    tile = const.tile([128, n * OSH], u8, name=f"cu{ci}")
                    nc.sync.dma_start(tile[:], src)
            for j, k in enumerate(kts):
                chunk_tiles[k] = tile[:, j * OSH : (j + 1) * OSH]

        # GPSIMD SWDGE ring (separate completion-sem space): xt + bias
        # bytes + scales-L1 head, then the cast chunks, interleaved with
        # the HWDGE issues so both streams start early.
        xt_sb = const.tile([128, (KT + 1) * B], f16)
        nc.gpsimd.dma_start(xt_sb[:], xt[:, :])
        bb_sb = const.tile([1, 4 * OSH + 2 * OSH], u8)
        nc.gpsimd.dma_start(bb_sb[:], bb[:, :])
        nc.gpsimd.dma_start(sc_sb[:, OSH : 2 * OSH], sc[:, OSH : 2 * OSH])
        # first code chunk BEFORE scales-L0: the ACT cast chain feeds the
        # first half of the DVE chain and must start earliest; DVE's first
        # multiply still starts ~3us earlier than with the full-sc order.
        _code_dma(0, HW_CHUNKS[0], swdge=False)
        nc.sync.dma_start(sc_sb[:, 0:OSH], sc[:, 0:OSH])
        for ci, kts in enumerate(HW_CHUNKS[1:], start=1):
            _code_dma(ci, kts, swdge=False)
        for ci, kts in enumerate(SW_CHUNKS):
            _code_dma(10 + ci, kts, swdge=True)
        bq_sb = bb_sb[0:1, 0 : 4 * OSH].bitcast(i32)
        bsx_sb = bb_sb[0:1, 4 * OSH : 6 * OSH].bitcast(f16)

        def sc_slice(kt):
            s = 0 if kt < NL0 else 1
            return sc_sb[:, s * OSH : (s + 1) * OSH]

        # --- small prologue ----------------------------------------------
        scr_d = const.tile([1, 8], f32)
        ones1 = const.tile([1, B], f16)
        nc.vector.memset(ones1[:], 1.0)
        # Touch sc halves + bias scales once on DVE so later DVE consumers'
        # waits are engine-order-covered (then stripped).
        nc.vector.tensor_copy(scr_d[0:1, 0:1], sc_sb[0:1, 0:1])
        nc.vector.tensor_copy(scr_d[0:1, 1:2], sc_sb[0:1, OSH : OSH + 1])
        nc.vector.tensor_copy(scr_d[0:1, 2:3], bsx_sb[0:1, 0:1])
        # bias tiles; the (bq-128) ACT cast is emitted mid-loop so it does
        # not block the ACT queue's head (costed ~4us of head when first)
        bias_q16 = const.tile([1, OSH], f16)
        bias16 = const.tile([1, OSH], f16)

        # PE wait-absorber for the one-time xt DMA (matmul LW struct carries
        # at most one sync wait).
        scrap = scrap_pool.tile([1, 4], f32)
        nc.tensor.matmul(
            scrap[0:1, 0:1], xt_sb[:, 0:1], xt_sb[:, 0:1], start=True, stop=True
        )

        # --- F-tile -128 correction lhsT ---------------------------------
        # fp32 reduce of the F tiles' x slices, exact hi+lo fp16 split.
        xsum16 = {}

        def _emit_xsum():
            for sel, kts in ((0, [k for k in CORR_KTS if k < NL0]),
                             (1, [k for k in CORR_KTS if k >= NL0])):
                if not kts:
                    continue
                acc = const.tile([128, B], f32, name=f"xsumf{sel}")
                spans = []
                for k in kts:
                    if spans and k == spans[-1][1]:
                        spans[-1] = (spans[-1][0], k + 1)
                    else:
                        spans.append((k, k + 1))
                first = True
                for (a, bnd) in spans:
                    n = bnd - a
                    view = xt_sb[:, a * B : bnd * B].rearrange(
                        "p (n b) -> p b n", n=n
                    )
                    if first:
                        nc.vector.tensor_reduce(
                            acc[:], view, mybir.AxisListType.X, mybir.AluOpType.add
                        )
                        first = False
                    else:
                        part = const.tile([128, B], f32, name=f"xsp{sel}_{a}")
                        nc.vector.tensor_reduce(
                            part[:], view, mybir.AxisListType.X, mybir.AluOpType.add
                        )
                        nc.vector.tensor_add(acc[:], acc[:], part[:])
                nc.vector.tensor_scalar_mul(acc[:], acc[:], -128.0)
                hi = const.tile([128, B], f16, name=f"xsumhi{sel}")
                nc.vector.tensor_copy(hi[:], acc[:])
                res = const.tile([128, B], f32, name=f"xsumr{sel}")
                nc.vector.tensor_tensor(
                    res[:], acc[:], hi[:], mybir.AluOpType.subtract
                )
                lo = const.tile([128, B], f16, name=f"xsumlo{sel}")
                nc.vector.tensor_copy(lo[:], res[:])
                xsum16[sel] = (hi, lo)

        # --- main pipeline ------------------------------------------------
        y_sb = ysb_pool.tile([B, OSH], f32)
        py = [py_pool.tile([B, 512], f32, name=f"py{g}") for g in range(NG)]

        for ei, kt in enumerate(EMIT_ORDER):
            if TYPES[kt] == "A":
                q16 = q16_pool.tile([128, OSH], f16)
                nc.scalar.activation(q16[:], chunk_tiles[kt], Copy, bias=-128.0)
                src = q16
            else:
                src = chunk_tiles[kt]
            wp = wp_pool.tile([128, OSH], f16)
            nc.vector.tensor_mul(wp[:], src[:], sc_slice(kt))
            for g in range(NG):
                nc.tensor.matmul(
                    py[g][:],
                    xt_sb[:, B * kt : B * (kt + 1)],
                    wp[:, 512 * g : 512 * (g + 1)],
                    start=ei == 0,
                    stop=False,
                )
            if ei == 2:
                _emit_xsum()
            if ei == 8:
                nc.scalar.activation(bias_q16[:], bq_sb, Copy, bias=-128.0)
            if ei == 5:
                for sel, (hi, lo) in xsum16.items():
                    for part in (hi, lo):
                        for g in range(NG):
                            nc.tensor.matmul(
                                py[g][:],
                                part[:],
                                sc_sb[:, sel * OSH + 512 * g : sel * OSH + 512 * (g + 1)],
                                start=False,
                                stop=False,
                            )
            if ei == 18:
                # bias16 = (bq-128) * bsx, fp16 2x TT; feeds the closing mms
                nc.vector.tensor_mul(bias16[:], bias_q16[:], bsx_sb)

        # bias via K=1 matmuls against the ones row, closing accumulation;
        # evacuation split ACT/DVE/ACT so the three banks drain in ~2 serial
        # copies instead of 3.
        for g in range(NG):
            nc.tensor.matmul(
                py[g][:],
                ones1[0:1, :],
                bias16[0:1, 512 * g : 512 * (g + 1)],
                start=False,
                stop=True,
            )
        for g in range(NG):
            nc.scalar.copy(y_sb[:, 512 * g : 512 * (g + 1)], py[g][:])
        nc.sync.dma_start(y[:, :], y_sb[:])

    _strip_self_waits(nc, mybir)
    return nc


_ENGINE_SEM_PREFIX = {
    "PE": "PE_",
    "DVE": "DVE_",
    "Activation": "Activation_",
    "SP": "SP_",
}


def _strip_self_waits(nc, mybir):
    """Several TRN2 ISA instruction structs encode at most ONE sync wait
    (walrus: "Too many sync wait commands").  Two classes of Tile-emitted
    waits are redundant and safe to drop from instructions carrying >=2:

    1. Self-engine waits: an engine completes its own instructions in order.
    2. Waits already observed (same value or higher) by an EARLIER
       instruction on the same in-order engine.

    Pool (GPSIMD) is special: the 8 Q7 cores do NOT complete in a single
    program order (so Pool_ self-sem waits are load-bearing and never
    dropped), but the Pool NX sequencer still DISPATCHES in order, and sem
    waits gate dispatch: a wait on an external sem already waited for by an
    earlier Pool instruction is dispatch-covered and droppable.

    Only monotonic sems (engine clocks, DMA lanes) may be deduped: barrier
    sems are reset by sem-subtract between rounds, so a repeated wait value
    there is NOT redundant (deduping one deadlocks the kernel).
    """
    fn = nc.m.functions[0]
    observed: dict = {}
    _MONO = ("DMAHW", "DMASW", "PE_", "DVE_", "Activation_", "SP_", "Pool_")

    def _dedupable(w):
        return w.ant_name.startswith(_MONO)

    for b in fn.blocks:
        for inst in b.instructions:
            si = inst.sync_info
            if si is None or not si.on_wait:
                continue
            eng = str(inst.engine)
            if eng.split(".")[-1] == "Pool":
                keep = [
                    w
                    for w in si.on_wait
                    if w.ant_name.startswith("Pool")
                    or not _dedupable(w)
                    or observed.get((eng, w.ant_name), 0) < w.wait_value
                ]
                for w in keep:
                    if _dedupable(w) and not w.ant_name.startswith("Pool"):
                        k = (eng, w.ant_name)
                        observed[k] = max(observed.get(k, 0), w.wait_value)
                if len(keep) != len(si.on_wait):
                    inst.sync_info = mybir.SyncInfo(
                        on_wait=keep, on_update=si.on_update
                    )
                continue
            if len(si.on_wait) < 2:
                for w in si.on_wait:
                    if _dedupable(w):
                        k = (eng, w.ant_name)
                        observed[k] = max(observed.get(k, 0), w.wait_value)
                continue
            keep = [
                w
                for w in si.on_wait
                if not _dedupable(w)
                or observed.get((eng, w.ant_name), 0) < w.wait_value
            ]
            pref = _ENGINE_SEM_PREFIX.get(str(inst.engine).split(".")[-1])
            if pref is not None:
                keep = [w for w in keep if not w.ant_name.startswith(pref)]
            if len(keep) >= 2 and type(inst).__name__ == "InstDMACopy":
                if any(
                    not w.ant_name.startswith(("DMAHW", "DMASW")) for w in keep
                ):
                    keep = [
                        w
                        for w in keep
                        if not w.ant_name.startswith(("DMAHW", "DMASW"))
                    ]
            for w in keep:
                if _dedupable(w):
                    k = (eng, w.ant_name)
                    observed[k] = max(observed.get(k, 0), w.wait_value)
            if len(keep) != len(si.on_wait):
                inst.sync_info = mybir.SyncInfo(
                    on_wait=keep, on_update=si.on_update
                )


def _get_nc():
    if "nc" not in _CACHE:
        _CACHE["nc"] = _build_nc()
    return _CACHE["nc"]


def _slot_permutation():
    """slot (kt, p) -> global i = 32*block + j.  16 L0 k-tiles map lane p to
    block p mod 96 (j = kt for p<96, 16+kt else); 8 L1 k-tiles map lane p to
    block 32 + p mod 64 (j = 16+g for p<64, 24+g else).  Bijective onto
    0..3071 (each (block, j) covered exactly once)."""
    i_slot = np.empty((KT, 128), dtype=np.int64)
    p = np.arange(128)
    for kt in range(NL0):
        b = np.where(p < 96, p, p - 96)
        j = np.where(p < 96, kt, 16 + kt)
        i_slot[kt] = 32 * b + j
    for g in range(KT - NL0):
        b = 32 + (p % 64)
        j = np.where(p < 64, 16 + g, 24 + g)
        i_slot[NL0 + g] = 32 * b + j
    return i_slot


def _make_in_maps(x, w_q, w_scales, b_q, b_scales):
    i_slot = _slot_permutation()
    flat = i_slot.reshape(-1)
    p = np.arange(128)
    r0_idx = np.where(p < 96, p, p - 96)
    r1_idx = 32 + (p % 64)

    x2 = np.ascontiguousarray(x.reshape(B, IN), dtype=np.float32)
    xtp = np.zeros((IN + 128, B), dtype=np.float16)               # [3200, 64]
    xtp[:IN] = x2[:, flat].T.astype(np.float16)
    xtp[IN] = 1.0
    # partition-major: [128, 25*64], row p = slot (kt, p) over all k-tiles
    xtp = np.ascontiguousarray(
        xtp.reshape(KT + 1, 128, B).transpose(1, 0, 2).reshape(128, (KT + 1) * B)
    )

    W8 = w_q.reshape(OUT, IN).astype(np.uint8)
    W8g = W8[:, flat]                                             # [OUT, 3072]
    ws_full = np.asarray(w_scales)                                # [12288, 96]
    bq_full = np.ascontiguousarray(b_q.reshape(OUT))
    bs_full = np.ascontiguousarray(b_scales)

    in_maps = []
    for c in range(NCORES):
        o0, o1 = c * OSH, (c + 1) * OSH
        allk = np.ascontiguousarray(
            W8g[o0:o1].T.reshape(KT, 128, OSH).transpose(1, 0, 2)
        )  # [128, KT, OSH]
        u8_order = [
            k
            for ch in ([c for c in HW_CHUNKS if TYPES[c[0]] == "A"] + SW_CHUNKS)
            for k in ch
        ]
        wqt_c = np.ascontiguousarray(
            allk[:, u8_order, :].reshape(128, len(u8_order) * OSH)
        )
        wqf_c = np.ascontiguousarray(
            allk[:, F_KTS, :].astype(np.float16).reshape(128, len(F_KTS) * OSH)
        )
        ws_c = ws_full[o0:o1].astype(np.float16)                  # [1536, 96]
        L0 = ws_c[:, r0_idx].T                                    # [128, 1536]
        L1 = ws_c[:, r1_idx].T
        sc_c = np.ascontiguousarray(np.concatenate([L0, L1], axis=1))
        bsx = np.repeat(
            bs_full[o0 // BLOCK : o1 // BLOCK].astype(np.float16), BLOCK
        )                                                         # [1536] f16
        bb_c = np.frombuffer(
            bq_full[o0:o1].astype("<i4").tobytes() + bsx.tobytes(),
            dtype=np.uint8,
        ).reshape(1, 6 * OSH)
        in_maps.append(
            {
                "wqt": wqt_c,
                "wqf": wqf_c,
                "sc": sc_c,
                "xt": xtp,
                "bb": bb_c,
            }
        )
    return in_maps


def run_shards(x, w_q, w_scales, b_q, b_scales, trace=False):
    """Run the SPMD kernel; returns (y_full, BassKernelResults)."""
    from concourse.bass_utils import run_bass_kernel_spmd

    nc = _get_nc()
    in_maps = _make_in_maps(x, w_q, w_scales, b_q, b_scales)
    res = run_bass_kernel_spmd(
        nc, in_maps, core_ids=list(range(NCORES)), trace=trace
    )
    shards = [np.asarray(res.results[c]["y"]) for c in range(NCORES)]
    y = np.concatenate(shards, axis=1).reshape(B, 1, OUT)
    return y, res


def kernel(**inputs):
    y, _ = run_shards(
        inputs["x"],
        inputs["w_q"],
        inputs["w_scales"],
        inputs["b_q"],
        inputs["b_scales"],
        trace=False,
    )
    return y.astype(np.float32)
